# revision 8
# baseline (speedup 1.0000x reference)
"""DiffEMA: 700-tap exponential-decay causal FIR over T=4194304 samples.

y[t] = sum_{k=0}^{K-1} alpha*(1-alpha)^k * x[t-k],  x[<0] := x[0]

Strategy: shard T across 8 cores (overlap-save: each core gets a 768-sample
left halo, host-sliced from the full input). Per core the convolution is cast
as 7 accumulating 128x128 matmuls per 512-column output tile:

  X[p, f] = x_chunk[f*128 + p]          (SBUF tile, 128 partitions x 4102 cols)
  Y[:, j] = sum_q C_q^T @ X[:, j+6-q]   (q = 0..6)
  C_q[pin, pout] = w[q*128 + pout - pin]  (0 outside [0, K))

The banded-Toeplitz matrices C_q are built host-side from w_alpha and
replicated to all cores.
"""

import math
import os

import numpy as np

import concourse.bass as bass
import concourse.bacc as bacc
import concourse.mybir as mybir
from concourse.tile import TileContext
from concourse.bass_utils import run_bass_kernel_spmd

T = 4194304
K = 700
N_CORES = 8
P = 128
S = T // N_CORES            # 524288 outputs per core
FCOL = S // P               # 4096 output columns per core
HALO_COLS = (K - 1 + P - 1) // P   # 6 halo columns = 768 samples >= K-1
HCOLS = FCOL + HALO_COLS    # 4102 input columns per core
NQ = HALO_COLS + 1          # 7 matmul taps
TILE_N = 512                # matmul moving free dim / one PSUM bank (fp32)
NTILES = FCOL // TILE_N     # 8 output tiles per core

DT = mybir.dt.float32

LAST_RESULT = None          # test harness introspection (exec_time_ns, trace)


XC_COLS = NQ * P + HCOLS   # packed [C | x] input: 896 + 4102 columns


def _build_nc():
    nc = bacc.Bacc()
    xc = nc.dram_tensor("xc", [P, XC_COLS], DT, kind="ExternalInput")
    y = nc.dram_tensor("y", [P, FCOL], DT, kind="ExternalOutput")
    XOFF = NQ * P

    with TileContext(nc) as tc:
        with (
            tc.tile_pool(name="xp", bufs=1) as xp,
            tc.tile_pool(name="ps", bufs=8, space="PSUM") as ps,
            tc.tile_pool(name="op", bufs=4) as op,
        ):
            xt = xp.tile([P, XC_COLS], DT)
            nc.sync.dma_start(out=xt[:, :], in_=xc[:, :])

            ot = None
            for t in range(NTILES):
                j0 = t * TILE_N
                acc = ps.tile([P, TILE_N], mybir.dt.float32)
                for q in range(NQ):
                    s0 = XOFF + j0 + HALO_COLS - q
                    nc.tensor.matmul(
                        acc[:, :],
                        lhsT=xt[:, q * P:(q + 1) * P],
                        rhs=xt[:, s0:s0 + TILE_N],
                        start=(q == 0),
                        stop=(q == NQ - 1),
                    )
                # pair up output tiles so only 4 out-DMAs are issued (the
                # 8 HWDGE sem lanes must not wrap onto the input DMA's lane:
                # a wrapped lane adds a 2nd sync-wait, which walrus rejects)
                if t % 2 == 0:
                    ot = op.tile([P, 2 * TILE_N], mybir.dt.float32)
                half = (t % 2) * TILE_N
                nc.vector.tensor_copy(out=ot[:, half:half + TILE_N], in_=acc[:, :])
                if t % 2 == 1:
                    nc.sync.dma_start(
                        out=y[:, j0 - TILE_N:j0 + TILE_N], in_=ot[:, :]
                    )
    return nc


def _build_cmat(w_alpha: float) -> np.ndarray:
    alpha = 1.0 / (1.0 + math.exp(-float(w_alpha)))
    k = np.arange(K, dtype=np.float64)
    w = (alpha * np.power(1.0 - alpha, k)).astype(np.float32)
    pin = np.arange(P)[:, None]
    pout = np.arange(P)[None, :]
    cmat = np.zeros((P, NQ * P), dtype=np.float32)
    for q in range(NQ):
        idx = q * P + pout - pin
        valid = (idx >= 0) & (idx < K)
        cmat[:, q * P:(q + 1) * P] = np.where(
            valid, w[np.clip(idx, 0, K - 1)], np.float32(0.0)
        )
    return cmat


def kernel(x, w_alpha):
    global LAST_RESULT
    x = np.asarray(x, dtype=np.float32).reshape(T)
    cmat = _build_cmat(np.asarray(w_alpha, dtype=np.float32))

    xg = np.concatenate([np.full(HALO_COLS * P, x[0], dtype=np.float32), x])
    in_maps = []
    for m in range(N_CORES):
        chunk = xg[m * S: m * S + S + HALO_COLS * P]
        xT = chunk.reshape(HCOLS, P).T
        xc = np.ascontiguousarray(np.concatenate([cmat, xT], axis=1))
        in_maps.append({"xc": xc})

    nc = _build_nc()
    nc.compile()
    res = run_bass_kernel_spmd(nc, in_maps, list(range(N_CORES)))
    LAST_RESULT = res

    out = np.empty((N_CORES, S), dtype=np.float32)
    for m in range(N_CORES):
        out[m] = res.results[m]["y"].T.reshape(-1)
    return out.reshape(T)


# revision 10
# speedup vs baseline: 1.9702x; 1.9702x over previous
"""DiffEMA: 700-tap exponential-decay causal FIR over T=4194304 samples.

y[t] = sum_{k=0}^{K-1} alpha*(1-alpha)^k * x[t-k],  x[<0] := x[0]

Strategy: shard T across 8 cores (overlap-save: each core gets a 768-sample
left halo, host-sliced from the full input). Per core the convolution is cast
as 7 accumulating 128x128 matmuls per 512-column output tile:

  X[p, f] = x_chunk[f*128 + p]          (SBUF tile, 128 partitions x 4102 cols)
  Y[:, j] = sum_q C_q^T @ X[:, j+6-q]   (q = 0..6)
  C_q[pin, pout] = w[q*128 + pout - pin]  (0 outside [0, K))

The banded-Toeplitz matrices C_q are built host-side from w_alpha and
replicated to all cores.
"""

import math
import os

import numpy as np

import concourse.bass as bass
import concourse.bacc as bacc
import concourse.mybir as mybir
from concourse.tile import TileContext
from concourse.bass_utils import run_bass_kernel_spmd

T = 4194304
K = 700
N_CORES = 8
P = 128
S = T // N_CORES            # 524288 outputs per core
FCOL = S // P               # 4096 output columns per core
HALO_COLS = (K - 1 + P - 1) // P   # 6 halo columns = 768 samples >= K-1
HCOLS = FCOL + HALO_COLS    # 4102 input columns per core
NQ = HALO_COLS + 1          # 7 matmul taps
TILE_N = 512                # matmul moving free dim / one PSUM bank (fp32)
NTILES = FCOL // TILE_N     # 8 output tiles per core

DT = mybir.dt.float32r

LAST_RESULT = None          # test harness introspection (exec_time_ns, trace)


XC_COLS = NQ * P + HCOLS   # packed [C | x] input: 896 + 4102 columns


def _build_nc():
    nc = bacc.Bacc()
    xc = nc.dram_tensor("xc", [P, XC_COLS], DT, kind="ExternalInput")
    y = nc.dram_tensor("y", [P, FCOL], mybir.dt.float32, kind="ExternalOutput")
    XOFF = NQ * P

    with TileContext(nc) as tc:
        with (
            tc.tile_pool(name="xp", bufs=1) as xp,
            tc.tile_pool(name="ps", bufs=8, space="PSUM") as ps,
            tc.tile_pool(name="op", bufs=4) as op,
        ):
            xt = xp.tile([P, XC_COLS], DT)
            nc.sync.dma_start(out=xt[:, :], in_=xc[:, :])

            ot = None
            for t in range(NTILES):
                j0 = t * TILE_N
                acc = ps.tile([P, TILE_N], mybir.dt.float32)
                for q in range(NQ):
                    s0 = XOFF + j0 + HALO_COLS - q
                    nc.tensor.matmul(
                        acc[:, :],
                        lhsT=xt[:, q * P:(q + 1) * P],
                        rhs=xt[:, s0:s0 + TILE_N],
                        start=(q == 0),
                        stop=(q == NQ - 1),
                    )
                # pair up output tiles so only 4 out-DMAs are issued (the
                # 8 HWDGE sem lanes must not wrap onto the input DMA's lane:
                # a wrapped lane adds a 2nd sync-wait, which walrus rejects)
                if t % 2 == 0:
                    ot = op.tile([P, 2 * TILE_N], mybir.dt.float32)
                half = (t % 2) * TILE_N
                nc.vector.tensor_copy(out=ot[:, half:half + TILE_N], in_=acc[:, :])
                if t % 2 == 1:
                    nc.sync.dma_start(
                        out=y[:, j0 - TILE_N:j0 + TILE_N], in_=ot[:, :]
                    )
    return nc


def _build_cmat(w_alpha: float) -> np.ndarray:
    alpha = 1.0 / (1.0 + math.exp(-float(w_alpha)))
    k = np.arange(K, dtype=np.float64)
    w = (alpha * np.power(1.0 - alpha, k)).astype(np.float32)
    pin = np.arange(P)[:, None]
    pout = np.arange(P)[None, :]
    cmat = np.zeros((P, NQ * P), dtype=np.float32)
    for q in range(NQ):
        idx = q * P + pout - pin
        valid = (idx >= 0) & (idx < K)
        cmat[:, q * P:(q + 1) * P] = np.where(
            valid, w[np.clip(idx, 0, K - 1)], np.float32(0.0)
        )
    return cmat


def kernel(x, w_alpha):
    global LAST_RESULT
    x = np.asarray(x, dtype=np.float32).reshape(T)
    cmat = _build_cmat(np.asarray(w_alpha, dtype=np.float32))

    xg = np.concatenate([np.full(HALO_COLS * P, x[0], dtype=np.float32), x])
    in_maps = []
    for m in range(N_CORES):
        chunk = xg[m * S: m * S + S + HALO_COLS * P]
        xT = chunk.reshape(HCOLS, P).T
        xc = np.ascontiguousarray(np.concatenate([cmat, xT], axis=1))
        in_maps.append({"xc": xc})

    nc = _build_nc()
    nc.compile()
    res = run_bass_kernel_spmd(nc, in_maps, list(range(N_CORES)))
    LAST_RESULT = res

    out = np.empty((N_CORES, S), dtype=np.float32)
    for m in range(N_CORES):
        out[m] = res.results[m]["y"].T.reshape(-1)
    return out.reshape(T)


# revision 11
# speedup vs baseline: 2.1231x; 1.0776x over previous
"""DiffEMA: 700-tap exponential-decay causal FIR over T=4194304 samples.

y[t] = sum_{k=0}^{K-1} alpha*(1-alpha)^k * x[t-k],  x[<0] := x[0]

Strategy: shard T across 8 cores (overlap-save: each core gets a 768-sample
left halo, host-sliced from the full input). Per core the convolution is cast
as 7 accumulating 128x128 matmuls per 512-column output tile:

  X[p, f] = x_chunk[f*128 + p]          (128 partitions, col-major samples)
  Y[:, j] = sum_q C_q^T @ X[:, j+6-q]   (q = 0..6)
  C_q[pin, pout] = w[q*128 + pout - pin]  (0 outside [0, K))

The banded-Toeplitz matrices C_q are built host-side from w_alpha and
replicated to all cores. Matmuls run in float32r (full PE rate for moving
free dim >= 256). The input is DMA'd in per-tile chunks so the PE starts
after the first ~270KB instead of after the full 2.1MB.
"""

import math

import numpy as np

import concourse.bacc as bacc
import concourse.mybir as mybir
from concourse.tile import TileContext
from concourse.bass_utils import run_bass_kernel_spmd

T = 4194304
K = 700
N_CORES = 8
P = 128
S = T // N_CORES            # 524288 outputs per core
FCOL = S // P               # 4096 output columns per core
HALO_COLS = (K - 1 + P - 1) // P   # 6 halo columns = 768 samples >= K-1
HCOLS = FCOL + HALO_COLS    # 4102 input columns per core
NQ = HALO_COLS + 1          # 7 matmul taps
TILE_N = 512                # matmul moving free dim / one PSUM bank (fp32)
NTILES = FCOL // TILE_N     # 8 output tiles per core
CHUNK = TILE_N + HALO_COLS  # input columns needed per output tile

DT = mybir.dt.float32r

LAST_RESULT = None          # test harness introspection (exec_time_ns, trace)


def _build_nc():
    nc = bacc.Bacc()
    c = nc.dram_tensor("c", [P, NQ * P], DT, kind="ExternalInput")
    x = nc.dram_tensor("x", [P, HCOLS], DT, kind="ExternalInput")
    y = nc.dram_tensor("y", [P, FCOL], mybir.dt.float32, kind="ExternalOutput")

    with TileContext(nc) as tc:
        with (
            tc.tile_pool(name="cp", bufs=1) as cp,
            tc.tile_pool(name="xp", bufs=3) as xp,
            tc.tile_pool(name="ps", bufs=8, space="PSUM") as ps,
            tc.tile_pool(name="op", bufs=4) as op,
        ):
            ct = cp.tile([P, NQ * P], DT)
            nc.sync.dma_start(out=ct[:, :], in_=c[:, :])

            ot = None
            for t in range(NTILES):
                j0 = t * TILE_N
                xt = xp.tile([P, CHUNK], DT)
                nc.sync.dma_start(out=xt[:, :], in_=x[:, j0:j0 + CHUNK])
                acc = ps.tile([P, TILE_N], mybir.dt.float32)
                for q in range(NQ):
                    nc.tensor.matmul(
                        acc[:, :],
                        lhsT=ct[:, q * P:(q + 1) * P],
                        rhs=xt[:, HALO_COLS - q:HALO_COLS - q + TILE_N],
                        start=(q == 0),
                        stop=(q == NQ - 1),
                    )
                # pair up output stores: fewer, larger DMAs
                if t % 2 == 0:
                    ot = op.tile([P, 2 * TILE_N], mybir.dt.float32)
                half = (t % 2) * TILE_N
                nc.vector.tensor_copy(out=ot[:, half:half + TILE_N], in_=acc[:, :])
                if t % 2 == 1:
                    nc.sync.dma_start(
                        out=y[:, j0 - TILE_N:j0 + TILE_N], in_=ot[:, :]
                    )
    return nc


def _build_cmat(w_alpha: float) -> np.ndarray:
    alpha = 1.0 / (1.0 + math.exp(-float(w_alpha)))
    k = np.arange(K, dtype=np.float64)
    w = (alpha * np.power(1.0 - alpha, k)).astype(np.float32)
    pin = np.arange(P)[:, None]
    pout = np.arange(P)[None, :]
    cmat = np.zeros((P, NQ * P), dtype=np.float32)
    for q in range(NQ):
        idx = q * P + pout - pin
        valid = (idx >= 0) & (idx < K)
        cmat[:, q * P:(q + 1) * P] = np.where(
            valid, w[np.clip(idx, 0, K - 1)], np.float32(0.0)
        )
    return cmat


def kernel(x, w_alpha):
    global LAST_RESULT
    x = np.asarray(x, dtype=np.float32).reshape(T)
    cmat = _build_cmat(np.asarray(w_alpha, dtype=np.float32))

    xg = np.concatenate([np.full(HALO_COLS * P, x[0], dtype=np.float32), x])
    in_maps = []
    for m in range(N_CORES):
        chunk = xg[m * S: m * S + S + HALO_COLS * P]
        xT = np.ascontiguousarray(chunk.reshape(HCOLS, P).T)
        in_maps.append({"x": xT, "c": cmat})

    nc = _build_nc()
    nc.compile()
    res = run_bass_kernel_spmd(nc, in_maps, list(range(N_CORES)))
    LAST_RESULT = res

    out = np.empty((N_CORES, S), dtype=np.float32)
    for m in range(N_CORES):
        out[m] = res.results[m]["y"].T.reshape(-1)
    return out.reshape(T)
